# revision 2
# baseline (speedup 1.0000x reference)
"""GraphSAGE 2-layer kernel for 8 trn2 NeuronCores (Bass/Tile), v3.

v4 = v3 plus a uint8-quantized h1 exchange: the AllGathered h1 table is
quantized per output-feature with host-derived scales (folded back into W2
on the host), halving the collective payload so the chunked AllGather hides
almost entirely under the layer-1 Pool work.

v3 = v2 (bf16 everywhere, transpose-via-matmul vs bf16 identity) plus an
ideal layer-1 sharding: each core computes h1 for a disjoint 12.5k-node
slice of the 100k layer-1 frontier (no cross-core recompute), then a
chunked AllGather replicates h1 to every core's DRAM for the layer-2
gathers.  This cuts the dominant cost — Pool-engine descriptor generation
for the per-slot indirect gathers (994ns/op, 128 rows/op) — from
189 tiles x 11 ops to 98 tiles x 11 ops per core.

The AllGather runs in 4 chunks so the first 3 overlap the tail of the
layer-1 compute; only the last chunk plus layer-2 is exposed.
"""

import os
import sys

os.environ.setdefault("NEURON_RT_RESET_CORES", "1")
sys.path.insert(0, "/opt/trn_rl_repo")
sys.path.insert(0, "/opt/pypackages")

import numpy as np

# Model dims (fixed by the problem)
N_RAW, N1, N2, K, D, OUT = 200000, 100000, 20000, 10, 256, 256
N_CORES = 8
P = 128
W = K + 1  # rows gathered per node: self + K neighbors

SH = N1 // N_CORES          # 12500 layer-1 nodes owned per core
T1 = -(-SH // P)            # 98 tiles
PAD_SH = T1 * P             # 12544 rows in the local h1 shard
NFULL = N_CORES * PAD_SH    # 100352 rows in the gathered h1 table
T2 = -(-(N2 // N_CORES) // P)  # 20 tiles
# AllGather chunk boundaries (in layer-1 tiles). Front-loaded: big early
# chunks ride the high-bandwidth regime and hide under remaining layer-1
# Pool work; the tiny last chunk minimizes the exposed serial gap before
# layer 2 can start.
AG_BOUNDS = [0, 32, 58, 74, 84, 92, T1]


def _build_program():
    import concourse.bacc as bacc
    import concourse.bass as bass
    import concourse.mybir as mybir
    import concourse.tile as tile
    from concourse.masks import make_identity

    f32 = mybir.dt.float32
    bf16 = mybir.dt.bfloat16
    i32 = mybir.dt.int32

    nc = bacc.Bacc(
        "TRN2", target_bir_lowering=False, debug=False, num_devices=N_CORES
    )

    rawf = nc.dram_tensor("rawf", [N_RAW, D], bf16, kind="ExternalInput")
    gidx1 = nc.dram_tensor("gidx1", [P, T1 * W], i32, kind="ExternalInput")
    gidx2 = nc.dram_tensor("gidx2", [P, T2 * W], i32, kind="ExternalInput")
    w1t = nc.dram_tensor("w1t", [2 * D, OUT], bf16, kind="ExternalInput")
    w2t = nc.dram_tensor("w2t", [2 * OUT, OUT], bf16, kind="ExternalInput")
    qsc = nc.dram_tensor("qsc", [P, OUT], f32, kind="ExternalInput")
    outd = nc.dram_tensor("out", [T2 * P, OUT], f32, kind="ExternalOutput")
    u8 = mybir.dt.uint8

    KC = 4               # contraction chunks (2*D/P = 2*OUT/P = 4)
    DC = D // P          # feature chunks per gathered row (2)

    with tile.TileContext(nc) as tc:
        with (
            tc.tile_pool(name="dram", bufs=1, space="DRAM") as dpool,
            tc.tile_pool(name="const", bufs=1) as cpool,
            tc.tile_pool(name="gbuf", bufs=33) as gpool,
            tc.tile_pool(name="xt", bufs=3) as xpool,
            tc.tile_pool(name="hout", bufs=4) as hpool,
            tc.tile_pool(name="pst", bufs=3, space="PSUM") as pspool,
            tc.tile_pool(name="pout", bufs=3, space="PSUM") as popool,
        ):
            h1s = dpool.tile([PAD_SH, OUT], u8)
            h1f = dpool.tile([NFULL, OUT], u8)

            ident = cpool.tile([P, P], bf16)
            make_identity(nc, ident[:])

            qsc_sb = cpool.tile([P, OUT], f32)
            nc.sync.dma_start(qsc_sb[:], qsc[:])

            idx1_sb = cpool.tile([P, T1 * W], i32)
            nc.sync.dma_start(idx1_sb[:], gidx1[:])
            idx2_sb = cpool.tile([P, T2 * W], i32)
            nc.sync.dma_start(idx2_sb[:], gidx2[:])

            w1_sb = cpool.tile([P, KC * OUT], bf16)
            for c in range(KC):
                nc.sync.dma_start(
                    w1_sb[:, c * OUT:(c + 1) * OUT], w1t[c * P:(c + 1) * P, :]
                )
            w2_sb = cpool.tile([P, KC * OUT], bf16)
            for c in range(KC):
                nc.sync.dma_start(
                    w2_sb[:, c * OUT:(c + 1) * OUT], w2t[c * P:(c + 1) * P, :]
                )

            def tile_step(i, idx_sb, table, w_sb, dst, out_dtype, gather_u8):
                # 11 single-index indirect gathers (HW honors one index per
                # partition), each into its own pool buffer. Layer 2 gathers
                # uint8 rows and casts them to bf16 on DVE for the PE.
                gs = []
                for k in range(W):
                    if gather_u8:
                        gq = gpool.tile([P, D], u8, tag="gq")
                        nc.gpsimd.indirect_dma_start(
                            out=gq[:],
                            out_offset=None,
                            in_=table,
                            in_offset=bass.IndirectOffsetOnAxis(
                                ap=idx_sb[:, i * W + k:i * W + k + 1], axis=0
                            ),
                        )
                        gk = gpool.tile([P, D], bf16, tag="g")
                        nc.vector.tensor_copy(gk[:], gq[:])
                    else:
                        gk = gpool.tile([P, D], bf16, tag="g")
                        nc.gpsimd.indirect_dma_start(
                            out=gk[:],
                            out_offset=None,
                            in_=table,
                            in_offset=bass.IndirectOffsetOnAxis(
                                ap=idx_sb[:, i * W + k:i * W + k + 1], axis=0
                            ),
                        )
                    gs.append(gk)
                # feature-major combined = [self ; sum_k neigh_k] via regular
                # matmuls against identity (bf16: 1 cycle/row, fp32 PSUM acc)
                psT = pspool.tile([P, 2 * D], f32, tag="psT")
                for c in range(DC):
                    nc.tensor.matmul(
                        out=psT[:, c * P:(c + 1) * P],
                        lhsT=gs[0][:, c * P:(c + 1) * P],
                        rhs=ident[:],
                        start=True,
                        stop=True,
                    )
                for c in range(DC):
                    o = D + c * P
                    for k in range(K):
                        nc.tensor.matmul(
                            out=psT[:, o:o + P],
                            lhsT=gs[1 + k][:, c * P:(c + 1) * P],
                            rhs=ident[:],
                            start=(k == 0),
                            stop=(k == K - 1),
                        )
                xT = xpool.tile([P, 2 * D], bf16, tag="xT")
                nc.scalar.copy(xT[:], psT[:])
                ph = popool.tile([P, OUT], f32, tag="ph")
                for c in range(KC):
                    nc.tensor.matmul(
                        out=ph[:],
                        lhsT=xT[:, c * P:(c + 1) * P],
                        rhs=w_sb[:, c * OUT:(c + 1) * OUT],
                        start=(c == 0),
                        stop=(c == KC - 1),
                    )
                if out_dtype == u8:
                    # relu, per-feature quant scale, clip, cast to uint8
                    hr = hpool.tile([P, OUT], f32, tag="hr")
                    nc.vector.tensor_scalar_max(hr[:], ph[:], 0.0)
                    hq = hpool.tile([P, OUT], f32, tag="hq")
                    nc.vector.tensor_tensor(
                        hq[:], hr[:], qsc_sb[:],
                        mybir.AluOpType.mult,
                    )
                    h = hpool.tile([P, OUT], u8, tag="h8")
                    nc.vector.tensor_scalar_min(h[:], hq[:], 254.99)
                else:
                    h = hpool.tile([P, OUT], out_dtype, tag="h")
                    nc.vector.tensor_scalar_max(h[:], ph[:], 0.0)
                nc.sync.dma_start(dst[i * P:(i + 1) * P, :], h[:])

            # Layer 1 over this core's shard, AllGather chunks interleaved
            # so they overlap the remaining layer-1 tiles.
            for lo, hi in zip(AG_BOUNDS[:-1], AG_BOUNDS[1:]):
                for i in range(lo, hi):
                    tile_step(i, idx1_sb, rawf[:], w1_sb, h1s[:], u8, False)
                lo_r, hi_r = lo * P, hi * P
                nc.gpsimd.collective_compute(
                    "AllGather",
                    bass.mybir.AluOpType.bypass,
                    replica_groups=[list(range(N_CORES))],
                    ins=[h1s[lo_r:hi_r, :].opt()],
                    outs=[
                        h1f[
                            N_CORES * lo_r:N_CORES * hi_r, :
                        ].opt()
                    ],
                )

            # Layer 2 against the gathered h1 table.
            for i in range(T2):
                tile_step(i, idx2_sb, h1f[:], w2_sb, outd[:], f32, True)

    nc.compile()
    return nc


def _prep_all(raw_features, W1, W2, nodes1, neigh1, self2, neigh2):
    """Host-side index prep. Returns in_maps (one per core)."""
    import ml_dtypes

    bf16 = ml_dtypes.bfloat16

    raw_b = np.ascontiguousarray(raw_features, dtype=np.float32).astype(bf16)
    W1 = np.asarray(W1, dtype=np.float32)
    W2 = np.asarray(W2, dtype=np.float32)
    nodes1 = np.asarray(nodes1, dtype=np.int32)
    neigh1 = np.asarray(neigh1, dtype=np.int32)
    self2 = np.asarray(self2, dtype=np.int32)
    neigh2 = np.asarray(neigh2, dtype=np.int32)

    # fold the 1/K neighbor-mean into the W columns that touch the agg half
    w1t = np.concatenate([W1[:, :D], W1[:, D:] / K], axis=1).T.copy().astype(bf16)

    # uint8 quantization of h1 for the AllGather: per-output-feature scale
    # 5 sigma of the (analytically known) pre-activation distribution.
    sig1 = np.sqrt(
        (W1[:, :D].astype(np.float64) ** 2).sum(1)
        + (W1[:, D:].astype(np.float64) ** 2).sum(1) / K
    )
    qscale = (255.0 / (5.0 * sig1)).astype(np.float32)  # h1 -> uint8 units
    # dequant folded into W2: both halves of W2's input are quantized h1
    deq = (1.0 / qscale.astype(np.float64))
    w2d = np.concatenate(
        [W2[:, :OUT] * deq[None, :], (W2[:, OUT:] / K) * deq[None, :]], axis=1
    )
    w2t = w2d.T.copy().astype(bf16)

    # global layer-1 node id -> row in the AllGathered h1 table. Each
    # AllGather chunk interleaves rank blocks: global row =
    # chunk_base * N_CORES + rank * chunk_rows + row_in_chunk.
    bounds_r = np.array(AG_BOUNDS) * P

    def h1f_row(n):
        rank, loc = n // SH, n % SH
        chunk = np.searchsorted(bounds_r, loc, side="right") - 1
        lo_r = bounds_r[chunk]
        hi_r = bounds_r[chunk + 1]
        return N_CORES * lo_r + rank * (hi_r - lo_r) + (loc - lo_r)

    in_maps = []
    n2_per = N2 // N_CORES
    for c in range(N_CORES):
        l1 = np.zeros((T1 * P, W), dtype=np.int32)
        sl = slice(c * SH, (c + 1) * SH)
        l1[:SH, 0] = nodes1[sl]
        l1[:SH, 1:] = neigh1[sl]

        l2 = np.zeros((T2 * P, W), dtype=np.int32)
        s2 = self2[c * n2_per:(c + 1) * n2_per]
        ng2 = neigh2[c * n2_per:(c + 1) * n2_per]
        l2[: s2.size, 0] = h1f_row(s2)
        l2[: ng2.shape[0], 1:] = h1f_row(ng2)

        in_maps.append(
            {
                "rawf": raw_b,
                "gidx1": _swizzle(l1, T1),
                "gidx2": _swizzle(l2, T2),
                "w1t": w1t,
                "w2t": w2t,
                "qsc": np.broadcast_to(qscale.reshape(1, OUT), (P, OUT)).copy(),
            }
        )
    return in_maps


def _swizzle(idx_rows, T):
    """[T*P, W] row-major -> [P, T*W] so tile i's idx = [:, i*W:(i+1)*W]."""
    return (
        idx_rows.reshape(T, P, W).transpose(1, 0, 2).reshape(P, T * W).copy()
    )


def kernel(raw_features, W1, W2, nodes1, neigh1, self2, neigh2, _trace=False):
    from concourse.bass_utils import run_bass_kernel_spmd

    in_maps = _prep_all(
        raw_features, W1, W2, nodes1, neigh1, self2, neigh2
    )
    nc = _build_program()
    res = run_bass_kernel_spmd(
        nc, in_maps, list(range(N_CORES)), trace=_trace
    )

    n_per = N2 // N_CORES
    out = np.concatenate(
        [res.results[c]["out"][:n_per] for c in range(N_CORES)], axis=0
    )
    if _trace:
        return out, res
    return out


# revision 5
# speedup vs baseline: 1.0904x; 1.0904x over previous
"""GraphSAGE 2-layer kernel for 8 trn2 NeuronCores (Bass/Tile), v5.

v5 = v4 plus referenced-only compaction: ~11% of layer-1 nodes are never
referenced by any layer-2 slot (self2/neigh2 are 220k draws over 100k), so
each core computes h1 only for the referenced nodes of its 12.5k-node
shard (compacted, padded to a uniform tile count), shrinking both the
layer-1 gather work and the AllGather payload.

v4 = v3 plus a uint8-quantized h1 exchange: the AllGathered h1 table is
quantized per output-feature with host-derived scales (folded back into W2
on the host), halving the collective payload so the chunked AllGather hides
almost entirely under the layer-1 Pool work.

v3 = v2 (bf16 everywhere, transpose-via-matmul vs bf16 identity) plus an
ideal layer-1 sharding: each core computes h1 for a disjoint 12.5k-node
slice of the 100k layer-1 frontier (no cross-core recompute), then a
chunked AllGather replicates h1 to every core's DRAM for the layer-2
gathers.  This cuts the dominant cost — Pool-engine descriptor generation
for the per-slot indirect gathers (994ns/op, 128 rows/op) — from
189 tiles x 11 ops to 98 tiles x 11 ops per core.

The AllGather runs in front-loaded chunks (AG_FRACS) so all but the tiny
last chunk hide under the layer-1 Pool work; only that last chunk plus
layer-2 is exposed.
"""

import os
import sys

os.environ.setdefault("NEURON_RT_RESET_CORES", "1")
sys.path.insert(0, "/opt/trn_rl_repo")
sys.path.insert(0, "/opt/pypackages")

import numpy as np

# Model dims (fixed by the problem)
N_RAW, N1, N2, K, D, OUT = 200000, 100000, 20000, 10, 256, 256
N_CORES = 8
P = 128
W = K + 1  # rows gathered per node: self + K neighbors

SH = N1 // N_CORES          # 12500 layer-1 nodes owned per core
T2 = -(-(N2 // N_CORES) // P)  # 20 tiles
# AllGather chunk boundaries as fractions of the layer-1 tile count.
# Front-loaded: big early chunks ride the high-bandwidth regime and hide
# under remaining layer-1 Pool work; the tiny last chunk minimizes the
# exposed serial gap before layer 2 can start.
AG_FRACS = (0.205, 0.5, 0.682, 0.795, 0.886, 0.955)


def _ag_bounds(T1):
    b = sorted({max(1, min(T1 - 1, round(f * T1))) for f in AG_FRACS})
    return [0] + b + [T1]


def _build_program(T1):
    PAD_SH = T1 * P
    NFULL = N_CORES * PAD_SH
    import concourse.bacc as bacc
    import concourse.bass as bass
    import concourse.mybir as mybir
    import concourse.tile as tile
    from concourse.masks import make_identity

    f32 = mybir.dt.float32
    bf16 = mybir.dt.bfloat16
    i32 = mybir.dt.int32

    nc = bacc.Bacc(
        "TRN2", target_bir_lowering=False, debug=False, num_devices=N_CORES
    )

    rawf = nc.dram_tensor("rawf", [N_RAW, D], bf16, kind="ExternalInput")
    gidx1 = nc.dram_tensor("gidx1", [P, T1 * W], i32, kind="ExternalInput")
    gidx2 = nc.dram_tensor("gidx2", [P, T2 * W], i32, kind="ExternalInput")
    w1t = nc.dram_tensor("w1t", [2 * D, OUT], bf16, kind="ExternalInput")
    w2t = nc.dram_tensor("w2t", [2 * OUT, OUT], bf16, kind="ExternalInput")
    qsc = nc.dram_tensor("qsc", [P, OUT], f32, kind="ExternalInput")
    outd = nc.dram_tensor("out", [T2 * P, OUT], f32, kind="ExternalOutput")
    u8 = mybir.dt.uint8

    KC = 4               # contraction chunks (2*D/P = 2*OUT/P = 4)
    DC = D // P          # feature chunks per gathered row (2)

    with tile.TileContext(nc) as tc:
        with (
            tc.tile_pool(name="dram", bufs=1, space="DRAM") as dpool,
            tc.tile_pool(name="const", bufs=1) as cpool,
            tc.tile_pool(name="gbuf", bufs=33) as gpool,
            tc.tile_pool(name="xt", bufs=3) as xpool,
            tc.tile_pool(name="hout", bufs=4) as hpool,
            tc.tile_pool(name="pst", bufs=3, space="PSUM") as pspool,
            tc.tile_pool(name="pout", bufs=3, space="PSUM") as popool,
        ):
            h1s = dpool.tile([PAD_SH, OUT], u8)
            h1f = dpool.tile([NFULL, OUT], u8)

            ident = cpool.tile([P, P], bf16)
            make_identity(nc, ident[:])

            qsc_sb = cpool.tile([P, OUT], f32)
            nc.sync.dma_start(qsc_sb[:], qsc[:])

            idx1_sb = cpool.tile([P, T1 * W], i32)
            nc.sync.dma_start(idx1_sb[:], gidx1[:])
            idx2_sb = cpool.tile([P, T2 * W], i32)
            nc.sync.dma_start(idx2_sb[:], gidx2[:])

            w1_sb = cpool.tile([P, KC * OUT], bf16)
            for c in range(KC):
                nc.sync.dma_start(
                    w1_sb[:, c * OUT:(c + 1) * OUT], w1t[c * P:(c + 1) * P, :]
                )
            w2_sb = cpool.tile([P, KC * OUT], bf16)
            for c in range(KC):
                nc.sync.dma_start(
                    w2_sb[:, c * OUT:(c + 1) * OUT], w2t[c * P:(c + 1) * P, :]
                )

            def tile_step(i, idx_sb, table, w_sb, dst, out_dtype, gather_u8):
                # 11 single-index indirect gathers (HW honors one index per
                # partition), each into its own pool buffer. Layer 2 gathers
                # uint8 rows and casts them to bf16 on DVE for the PE.
                gs = []
                for k in range(W):
                    if gather_u8:
                        gq = gpool.tile([P, D], u8, tag="gq")
                        nc.gpsimd.indirect_dma_start(
                            out=gq[:],
                            out_offset=None,
                            in_=table,
                            in_offset=bass.IndirectOffsetOnAxis(
                                ap=idx_sb[:, i * W + k:i * W + k + 1], axis=0
                            ),
                        )
                        gk = gpool.tile([P, D], bf16, tag="g")
                        nc.vector.tensor_copy(gk[:], gq[:])
                    else:
                        gk = gpool.tile([P, D], bf16, tag="g")
                        nc.gpsimd.indirect_dma_start(
                            out=gk[:],
                            out_offset=None,
                            in_=table,
                            in_offset=bass.IndirectOffsetOnAxis(
                                ap=idx_sb[:, i * W + k:i * W + k + 1], axis=0
                            ),
                        )
                    gs.append(gk)
                # feature-major combined = [self ; sum_k neigh_k] via regular
                # matmuls against identity (bf16: 1 cycle/row, fp32 PSUM acc)
                psT = pspool.tile([P, 2 * D], f32, tag="psT")
                for c in range(DC):
                    nc.tensor.matmul(
                        out=psT[:, c * P:(c + 1) * P],
                        lhsT=gs[0][:, c * P:(c + 1) * P],
                        rhs=ident[:],
                        start=True,
                        stop=True,
                    )
                for c in range(DC):
                    o = D + c * P
                    for k in range(K):
                        nc.tensor.matmul(
                            out=psT[:, o:o + P],
                            lhsT=gs[1 + k][:, c * P:(c + 1) * P],
                            rhs=ident[:],
                            start=(k == 0),
                            stop=(k == K - 1),
                        )
                xT = xpool.tile([P, 2 * D], bf16, tag="xT")
                nc.scalar.copy(xT[:], psT[:])
                ph = popool.tile([P, OUT], f32, tag="ph")
                for c in range(KC):
                    nc.tensor.matmul(
                        out=ph[:],
                        lhsT=xT[:, c * P:(c + 1) * P],
                        rhs=w_sb[:, c * OUT:(c + 1) * OUT],
                        start=(c == 0),
                        stop=(c == KC - 1),
                    )
                if out_dtype == u8:
                    # relu, per-feature quant scale, clip, cast to uint8
                    hr = hpool.tile([P, OUT], f32, tag="hr")
                    nc.vector.tensor_scalar_max(hr[:], ph[:], 0.0)
                    hq = hpool.tile([P, OUT], f32, tag="hq")
                    nc.vector.tensor_tensor(
                        hq[:], hr[:], qsc_sb[:],
                        mybir.AluOpType.mult,
                    )
                    h = hpool.tile([P, OUT], u8, tag="h8")
                    nc.vector.tensor_scalar_min(h[:], hq[:], 254.99)
                else:
                    h = hpool.tile([P, OUT], out_dtype, tag="h")
                    nc.vector.tensor_scalar_max(h[:], ph[:], 0.0)
                nc.sync.dma_start(dst[i * P:(i + 1) * P, :], h[:])

            # Layer 1 over this core's shard, AllGather chunks interleaved
            # so they overlap the remaining layer-1 tiles.
            bounds = _ag_bounds(T1)
            for lo, hi in zip(bounds[:-1], bounds[1:]):
                for i in range(lo, hi):
                    tile_step(i, idx1_sb, rawf[:], w1_sb, h1s[:], u8, False)
                lo_r, hi_r = lo * P, hi * P
                nc.gpsimd.collective_compute(
                    "AllGather",
                    bass.mybir.AluOpType.bypass,
                    replica_groups=[list(range(N_CORES))],
                    ins=[h1s[lo_r:hi_r, :].opt()],
                    outs=[
                        h1f[
                            N_CORES * lo_r:N_CORES * hi_r, :
                        ].opt()
                    ],
                )

            # Layer 2 against the gathered h1 table.
            for i in range(T2):
                tile_step(i, idx2_sb, h1f[:], w2_sb, outd[:], f32, True)

    nc.compile()
    return nc


def _prep_all(raw_features, W1, W2, nodes1, neigh1, self2, neigh2):
    """Host-side index prep. Returns in_maps (one per core)."""
    import ml_dtypes

    bf16 = ml_dtypes.bfloat16

    raw_b = np.ascontiguousarray(raw_features, dtype=np.float32).astype(bf16)
    W1 = np.asarray(W1, dtype=np.float32)
    W2 = np.asarray(W2, dtype=np.float32)
    nodes1 = np.asarray(nodes1, dtype=np.int32)
    neigh1 = np.asarray(neigh1, dtype=np.int32)
    self2 = np.asarray(self2, dtype=np.int32)
    neigh2 = np.asarray(neigh2, dtype=np.int32)

    # fold the 1/K neighbor-mean into the W columns that touch the agg half
    w1t = np.concatenate([W1[:, :D], W1[:, D:] / K], axis=1).T.copy().astype(bf16)

    # uint8 quantization of h1 for the AllGather: per-output-feature scale
    # 5 sigma of the (analytically known) pre-activation distribution.
    sig1 = np.sqrt(
        (W1[:, :D].astype(np.float64) ** 2).sum(1)
        + (W1[:, D:].astype(np.float64) ** 2).sum(1) / K
    )
    qscale = (255.0 / (5.0 * sig1)).astype(np.float32)  # h1 -> uint8 units
    # dequant folded into W2: both halves of W2's input are quantized h1
    deq = (1.0 / qscale.astype(np.float64))
    w2d = np.concatenate(
        [W2[:, :OUT] * deq[None, :], (W2[:, OUT:] / K) * deq[None, :]], axis=1
    )
    w2t = w2d.T.copy().astype(bf16)

    # Compact each rank's shard to the layer-1 nodes actually referenced by
    # layer 2 (identical on every core: derived from the full self2/neigh2).
    ref = np.unique(np.concatenate([self2, neigh2.ravel()]))
    rank_of = ref // SH
    counts = np.bincount(rank_of, minlength=N_CORES)
    T1 = int(-(-counts.max() // P))
    # rank-local compact position of each referenced node
    order = np.argsort(rank_of, kind="stable")
    pos_in_rank = np.empty(ref.size, dtype=np.int32)
    start = 0
    for c in range(N_CORES):
        pos_in_rank[order[start:start + counts[c]]] = np.arange(
            counts[c], dtype=np.int32
        )
        start += counts[c]
    loc_map = np.zeros(N1, dtype=np.int32)
    loc_map[ref] = pos_in_rank

    # global layer-1 node id -> row in the AllGathered h1 table. Each
    # AllGather chunk interleaves rank blocks: global row =
    # chunk_base * N_CORES + rank * chunk_rows + row_in_chunk.
    bounds_r = np.array(_ag_bounds(T1)) * P

    def h1f_row(n):
        rank, loc = n // SH, loc_map[n]
        chunk = np.searchsorted(bounds_r, loc, side="right") - 1
        lo_r = bounds_r[chunk]
        hi_r = bounds_r[chunk + 1]
        return N_CORES * lo_r + rank * (hi_r - lo_r) + (loc - lo_r)

    in_maps = []
    n2_per = N2 // N_CORES
    for c in range(N_CORES):
        l1 = np.zeros((T1 * P, W), dtype=np.int32)
        ref_c = ref[rank_of == c]
        l1[: ref_c.size, 0] = nodes1[ref_c]
        l1[: ref_c.size, 1:] = neigh1[ref_c]

        l2 = np.zeros((T2 * P, W), dtype=np.int32)
        s2 = self2[c * n2_per:(c + 1) * n2_per]
        ng2 = neigh2[c * n2_per:(c + 1) * n2_per]
        l2[: s2.size, 0] = h1f_row(s2)
        l2[: ng2.shape[0], 1:] = h1f_row(ng2)

        in_maps.append(
            {
                "rawf": raw_b,
                "gidx1": _swizzle(l1, T1),
                "gidx2": _swizzle(l2, T2),
                "w1t": w1t,
                "w2t": w2t,
                "qsc": np.broadcast_to(qscale.reshape(1, OUT), (P, OUT)).copy(),
            }
        )
    return T1, in_maps


def _swizzle(idx_rows, T):
    """[T*P, W] row-major -> [P, T*W] so tile i's idx = [:, i*W:(i+1)*W]."""
    return (
        idx_rows.reshape(T, P, W).transpose(1, 0, 2).reshape(P, T * W).copy()
    )


def kernel(raw_features, W1, W2, nodes1, neigh1, self2, neigh2, _trace=False):
    from concourse.bass_utils import run_bass_kernel_spmd

    T1, in_maps = _prep_all(
        raw_features, W1, W2, nodes1, neigh1, self2, neigh2
    )
    nc = _build_program(T1)
    res = run_bass_kernel_spmd(
        nc, in_maps, list(range(N_CORES)), trace=_trace
    )

    n_per = N2 // N_CORES
    out = np.concatenate(
        [res.results[c]["out"][:n_per] for c in range(N_CORES)], axis=0
    )
    if _trace:
        return out, res
    return out


# revision 6
# speedup vs baseline: 1.1032x; 1.0117x over previous
"""GraphSAGE 2-layer kernel for 8 trn2 NeuronCores (Bass/Tile), v5.

v6 = v5 plus: balanced rank assignment of the referenced layer-1 nodes
(11104/core -> 87 tiles instead of 88), and each AllGather is emitted one
tile later in Pool program order so its input-ready wait never stalls the
Pool sequencer between gather ops.

v5 = v4 plus referenced-only compaction: ~11% of layer-1 nodes are never
referenced by any layer-2 slot (self2/neigh2 are 220k draws over 100k), so
each core computes h1 only for the referenced nodes of its 12.5k-node
shard (compacted, padded to a uniform tile count), shrinking both the
layer-1 gather work and the AllGather payload.

v4 = v3 plus a uint8-quantized h1 exchange: the AllGathered h1 table is
quantized per output-feature with host-derived scales (folded back into W2
on the host), halving the collective payload so the chunked AllGather hides
almost entirely under the layer-1 Pool work.

v3 = v2 (bf16 everywhere, transpose-via-matmul vs bf16 identity) plus an
ideal layer-1 sharding: each core computes h1 for a disjoint 12.5k-node
slice of the 100k layer-1 frontier (no cross-core recompute), then a
chunked AllGather replicates h1 to every core's DRAM for the layer-2
gathers.  This cuts the dominant cost — Pool-engine descriptor generation
for the per-slot indirect gathers (994ns/op, 128 rows/op) — from
189 tiles x 11 ops to 98 tiles x 11 ops per core.

The AllGather runs in front-loaded chunks (AG_FRACS) so all but the tiny
last chunk hide under the layer-1 Pool work; only that last chunk plus
layer-2 is exposed.
"""

import os
import sys

os.environ.setdefault("NEURON_RT_RESET_CORES", "1")
sys.path.insert(0, "/opt/trn_rl_repo")
sys.path.insert(0, "/opt/pypackages")

import numpy as np

# Model dims (fixed by the problem)
N_RAW, N1, N2, K, D, OUT = 200000, 100000, 20000, 10, 256, 256
N_CORES = 8
P = 128
W = K + 1  # rows gathered per node: self + K neighbors

SH = N1 // N_CORES          # 12500 layer-1 nodes owned per core
T2 = -(-(N2 // N_CORES) // P)  # 20 tiles
# AllGather chunk boundaries as fractions of the layer-1 tile count.
# Front-loaded: big early chunks ride the high-bandwidth regime and hide
# under remaining layer-1 Pool work; the tiny last chunk minimizes the
# exposed serial gap before layer 2 can start.
AG_FRACS = (0.184, 0.483, 0.667, 0.782, 0.874, 0.943)


def _ag_bounds(T1):
    b = sorted({max(1, min(T1 - 1, round(f * T1))) for f in AG_FRACS})
    return [0] + b + [T1]


def _build_program(T1):
    PAD_SH = T1 * P
    NFULL = N_CORES * PAD_SH
    import concourse.bacc as bacc
    import concourse.bass as bass
    import concourse.mybir as mybir
    import concourse.tile as tile
    from concourse.masks import make_identity

    f32 = mybir.dt.float32
    bf16 = mybir.dt.bfloat16
    i32 = mybir.dt.int32

    nc = bacc.Bacc(
        "TRN2", target_bir_lowering=False, debug=False, num_devices=N_CORES
    )

    rawf = nc.dram_tensor("rawf", [N_RAW, D], bf16, kind="ExternalInput")
    gidx1 = nc.dram_tensor("gidx1", [P, T1 * W], i32, kind="ExternalInput")
    gidx2 = nc.dram_tensor("gidx2", [P, T2 * W], i32, kind="ExternalInput")
    w1t = nc.dram_tensor("w1t", [2 * D, OUT], bf16, kind="ExternalInput")
    w2t = nc.dram_tensor("w2t", [2 * OUT, OUT], bf16, kind="ExternalInput")
    qsc = nc.dram_tensor("qsc", [P, OUT], f32, kind="ExternalInput")
    outd = nc.dram_tensor("out", [T2 * P, OUT], f32, kind="ExternalOutput")
    u8 = mybir.dt.uint8

    KC = 4               # contraction chunks (2*D/P = 2*OUT/P = 4)
    DC = D // P          # feature chunks per gathered row (2)

    with tile.TileContext(nc) as tc:
        with (
            tc.tile_pool(name="dram", bufs=1, space="DRAM") as dpool,
            tc.tile_pool(name="const", bufs=1) as cpool,
            tc.tile_pool(name="gbuf", bufs=33) as gpool,
            tc.tile_pool(name="xt", bufs=3) as xpool,
            tc.tile_pool(name="hout", bufs=4) as hpool,
            tc.tile_pool(name="pst", bufs=3, space="PSUM") as pspool,
            tc.tile_pool(name="pout", bufs=3, space="PSUM") as popool,
        ):
            h1s = dpool.tile([PAD_SH, OUT], u8)
            h1f = dpool.tile([NFULL, OUT], u8)

            ident = cpool.tile([P, P], bf16)
            make_identity(nc, ident[:])

            qsc_sb = cpool.tile([P, OUT], f32)
            nc.sync.dma_start(qsc_sb[:], qsc[:])

            idx1_sb = cpool.tile([P, T1 * W], i32)
            nc.sync.dma_start(idx1_sb[:], gidx1[:])
            idx2_sb = cpool.tile([P, T2 * W], i32)
            nc.sync.dma_start(idx2_sb[:], gidx2[:])

            w1_sb = cpool.tile([P, KC * OUT], bf16)
            for c in range(KC):
                nc.sync.dma_start(
                    w1_sb[:, c * OUT:(c + 1) * OUT], w1t[c * P:(c + 1) * P, :]
                )
            w2_sb = cpool.tile([P, KC * OUT], bf16)
            for c in range(KC):
                nc.sync.dma_start(
                    w2_sb[:, c * OUT:(c + 1) * OUT], w2t[c * P:(c + 1) * P, :]
                )

            def tile_step(i, idx_sb, table, w_sb, dst, out_dtype, gather_u8):
                # 11 single-index indirect gathers (HW honors one index per
                # partition), each into its own pool buffer. Layer 2 gathers
                # uint8 rows and casts them to bf16 on DVE for the PE.
                gs = []
                for k in range(W):
                    if gather_u8:
                        gq = gpool.tile([P, D], u8, tag="gq")
                        nc.gpsimd.indirect_dma_start(
                            out=gq[:],
                            out_offset=None,
                            in_=table,
                            in_offset=bass.IndirectOffsetOnAxis(
                                ap=idx_sb[:, i * W + k:i * W + k + 1], axis=0
                            ),
                        )
                        gk = gpool.tile([P, D], bf16, tag="g")
                        nc.vector.tensor_copy(gk[:], gq[:])
                    else:
                        gk = gpool.tile([P, D], bf16, tag="g")
                        nc.gpsimd.indirect_dma_start(
                            out=gk[:],
                            out_offset=None,
                            in_=table,
                            in_offset=bass.IndirectOffsetOnAxis(
                                ap=idx_sb[:, i * W + k:i * W + k + 1], axis=0
                            ),
                        )
                    gs.append(gk)
                # feature-major combined = [self ; sum_k neigh_k] via regular
                # matmuls against identity (bf16: 1 cycle/row, fp32 PSUM acc)
                psT = pspool.tile([P, 2 * D], f32, tag="psT")
                for c in range(DC):
                    nc.tensor.matmul(
                        out=psT[:, c * P:(c + 1) * P],
                        lhsT=gs[0][:, c * P:(c + 1) * P],
                        rhs=ident[:],
                        start=True,
                        stop=True,
                    )
                for c in range(DC):
                    o = D + c * P
                    for k in range(K):
                        nc.tensor.matmul(
                            out=psT[:, o:o + P],
                            lhsT=gs[1 + k][:, c * P:(c + 1) * P],
                            rhs=ident[:],
                            start=(k == 0),
                            stop=(k == K - 1),
                        )
                xT = xpool.tile([P, 2 * D], bf16, tag="xT")
                nc.scalar.copy(xT[:], psT[:])
                ph = popool.tile([P, OUT], f32, tag="ph")
                for c in range(KC):
                    nc.tensor.matmul(
                        out=ph[:],
                        lhsT=xT[:, c * P:(c + 1) * P],
                        rhs=w_sb[:, c * OUT:(c + 1) * OUT],
                        start=(c == 0),
                        stop=(c == KC - 1),
                    )
                if out_dtype == u8:
                    # relu, per-feature quant scale, clip, cast to uint8
                    hr = hpool.tile([P, OUT], f32, tag="hr")
                    nc.vector.tensor_scalar_max(hr[:], ph[:], 0.0)
                    hq = hpool.tile([P, OUT], f32, tag="hq")
                    nc.vector.tensor_tensor(
                        hq[:], hr[:], qsc_sb[:],
                        mybir.AluOpType.mult,
                    )
                    h = hpool.tile([P, OUT], u8, tag="h8")
                    nc.vector.tensor_scalar_min(h[:], hq[:], 254.99)
                else:
                    h = hpool.tile([P, OUT], out_dtype, tag="h")
                    nc.vector.tensor_scalar_max(h[:], ph[:], 0.0)
                nc.sync.dma_start(dst[i * P:(i + 1) * P, :], h[:])

            # Layer 1 over this core's shard, AllGather chunks interleaved
            # so they overlap the remaining layer-1 tiles. Each AG is emitted
            # one tile AFTER its input range completes: by then the h1s
            # stores have long landed, so the AG's wait is already satisfied
            # and never stalls the Pool sequencer between gather ops.
            bounds = _ag_bounds(T1)
            pending = list(zip(bounds[:-1], bounds[1:]))

            def emit_ag():
                lo, hi = pending.pop(0)
                lo_r, hi_r = lo * P, hi * P
                nc.gpsimd.collective_compute(
                    "AllGather",
                    bass.mybir.AluOpType.bypass,
                    replica_groups=[list(range(N_CORES))],
                    ins=[h1s[lo_r:hi_r, :].opt()],
                    outs=[h1f[N_CORES * lo_r:N_CORES * hi_r, :].opt()],
                )

            for i in range(T1):
                tile_step(i, idx1_sb, rawf[:], w1_sb, h1s[:], u8, False)
                if pending and i >= pending[0][1]:
                    emit_ag()
            while pending:
                emit_ag()

            # Layer 2 against the gathered h1 table.
            for i in range(T2):
                tile_step(i, idx2_sb, h1f[:], w2_sb, outd[:], f32, True)

    nc.compile()
    return nc


def _prep_all(raw_features, W1, W2, nodes1, neigh1, self2, neigh2):
    """Host-side index prep. Returns in_maps (one per core)."""
    import ml_dtypes

    bf16 = ml_dtypes.bfloat16

    raw_b = np.ascontiguousarray(raw_features, dtype=np.float32).astype(bf16)
    W1 = np.asarray(W1, dtype=np.float32)
    W2 = np.asarray(W2, dtype=np.float32)
    nodes1 = np.asarray(nodes1, dtype=np.int32)
    neigh1 = np.asarray(neigh1, dtype=np.int32)
    self2 = np.asarray(self2, dtype=np.int32)
    neigh2 = np.asarray(neigh2, dtype=np.int32)

    # fold the 1/K neighbor-mean into the W columns that touch the agg half
    w1t = np.concatenate([W1[:, :D], W1[:, D:] / K], axis=1).T.copy().astype(bf16)

    # uint8 quantization of h1 for the AllGather: per-output-feature scale
    # 5 sigma of the (analytically known) pre-activation distribution.
    sig1 = np.sqrt(
        (W1[:, :D].astype(np.float64) ** 2).sum(1)
        + (W1[:, D:].astype(np.float64) ** 2).sum(1) / K
    )
    qscale = (255.0 / (5.0 * sig1)).astype(np.float32)  # h1 -> uint8 units
    # dequant folded into W2: both halves of W2's input are quantized h1
    deq = (1.0 / qscale.astype(np.float64))
    w2d = np.concatenate(
        [W2[:, :OUT] * deq[None, :], (W2[:, OUT:] / K) * deq[None, :]], axis=1
    )
    w2t = w2d.T.copy().astype(bf16)

    # Compact each rank's shard to the layer-1 nodes actually referenced by
    # layer 2 (identical on every core: derived from the full self2/neigh2).
    ref = np.unique(np.concatenate([self2, neigh2.ravel()]))
    per = -(-ref.size // N_CORES)
    rank_of = np.minimum(np.arange(ref.size) // per, N_CORES - 1).astype(
        np.int32
    )
    counts = np.bincount(rank_of, minlength=N_CORES)
    T1 = int(-(-counts.max() // P))
    rank_arr = np.zeros(N1, dtype=np.int32)
    rank_arr[ref] = rank_of
    # rank-local compact position of each referenced node
    order = np.argsort(rank_of, kind="stable")
    pos_in_rank = np.empty(ref.size, dtype=np.int32)
    start = 0
    for c in range(N_CORES):
        pos_in_rank[order[start:start + counts[c]]] = np.arange(
            counts[c], dtype=np.int32
        )
        start += counts[c]
    loc_map = np.zeros(N1, dtype=np.int32)
    loc_map[ref] = pos_in_rank

    # global layer-1 node id -> row in the AllGathered h1 table. Each
    # AllGather chunk interleaves rank blocks: global row =
    # chunk_base * N_CORES + rank * chunk_rows + row_in_chunk.
    bounds_r = np.array(_ag_bounds(T1)) * P

    def h1f_row(n):
        rank, loc = rank_arr[n], loc_map[n]
        chunk = np.searchsorted(bounds_r, loc, side="right") - 1
        lo_r = bounds_r[chunk]
        hi_r = bounds_r[chunk + 1]
        return N_CORES * lo_r + rank * (hi_r - lo_r) + (loc - lo_r)

    in_maps = []
    n2_per = N2 // N_CORES
    for c in range(N_CORES):
        l1 = np.zeros((T1 * P, W), dtype=np.int32)
        ref_c = ref[rank_of == c]
        l1[: ref_c.size, 0] = nodes1[ref_c]
        l1[: ref_c.size, 1:] = neigh1[ref_c]

        l2 = np.zeros((T2 * P, W), dtype=np.int32)
        s2 = self2[c * n2_per:(c + 1) * n2_per]
        ng2 = neigh2[c * n2_per:(c + 1) * n2_per]
        l2[: s2.size, 0] = h1f_row(s2)
        l2[: ng2.shape[0], 1:] = h1f_row(ng2)

        in_maps.append(
            {
                "rawf": raw_b,
                "gidx1": _swizzle(l1, T1),
                "gidx2": _swizzle(l2, T2),
                "w1t": w1t,
                "w2t": w2t,
                "qsc": np.broadcast_to(qscale.reshape(1, OUT), (P, OUT)).copy(),
            }
        )
    return T1, in_maps


def _swizzle(idx_rows, T):
    """[T*P, W] row-major -> [P, T*W] so tile i's idx = [:, i*W:(i+1)*W]."""
    return (
        idx_rows.reshape(T, P, W).transpose(1, 0, 2).reshape(P, T * W).copy()
    )


def kernel(raw_features, W1, W2, nodes1, neigh1, self2, neigh2, _trace=False):
    from concourse.bass_utils import run_bass_kernel_spmd

    T1, in_maps = _prep_all(
        raw_features, W1, W2, nodes1, neigh1, self2, neigh2
    )
    nc = _build_program(T1)
    res = run_bass_kernel_spmd(
        nc, in_maps, list(range(N_CORES)), trace=_trace
    )

    n_per = N2 // N_CORES
    out = np.concatenate(
        [res.results[c]["out"][:n_per] for c in range(N_CORES)], axis=0
    )
    if _trace:
        return out, res
    return out
